# revision 11
# baseline (speedup 1.0000x reference)
"""CrossAttention (cosine-normalized QK) Trainium2 Bass kernel, 8-core SPMD.

Sharding: batch (2) x query-row blocks (4) -> 8 cores. Each core computes the
full K/V projection for its batch (replicated within a batch group) and a
512-row slice of queries; output rows are disjoint, so the gather is a pure
concatenation (no collectives).

v5: input DMAs prioritized per consuming phase and spread across engine
queues, exp split between ScalarE (LUT) and VectorE (degree-4 polynomial,
max err 1e-4 on |s|<=0.125), attention rowsum-normalize muls moved to GpSimd
so the DVE queue never blocks on the DMA reciprocal round-trips.
"""

import numpy as np
import ml_dtypes
from contextlib import ExitStack

import concourse.bacc as bacc
import concourse.bass as bass
import concourse.mybir as mybir
import concourse.tile as tile
from concourse import bass_utils

F32 = mybir.dt.float32
BF16 = mybir.dt.bfloat16
AF = mybir.ActivationFunctionType
ALU = mybir.AluOpType

B, NQ, NK = 2, 2048, 2048
QD, KD, E, H = 1024, 768, 1024, 16
D = E // H          # 64
NC = 8              # cores
NQC = NQ * B // NC  # 512 query rows per core
SCALE = D ** -0.5   # 0.125
LN_EPS = 1e-5

IC_Q = QD // 128    # 8  contraction chunks for Q proj
IC_K = KD // 128    # 6  contraction chunks for K/V proj
EC = E // 128       # 8  embed chunks
KC = NK // 128      # 16 key chunks
NT = NQC // 128     # 4  query-row tiles
HP = H // 2         # 8  head pairs
KS = 4              # key super-blocks (512 keys each)


def build():
    nc = bacc.Bacc("TRN2", target_bir_lowering=False, debug=False,
                   enable_asserts=False, num_devices=1)

    qT = nc.dram_tensor("qT", [QD, NQC], BF16, kind="ExternalInput").ap()
    kT = nc.dram_tensor("kT", [KD, NK], BF16, kind="ExternalInput").ap()
    vT = nc.dram_tensor("vT", [KD, NK], BF16, kind="ExternalInput").ap()
    wq = nc.dram_tensor("wq", [QD, E], BF16, kind="ExternalInput").ap()
    wk = nc.dram_tensor("wk", [KD, E], BF16, kind="ExternalInput").ap()
    wv = nc.dram_tensor("wv", [KD, E], BF16, kind="ExternalInput").ap()
    wo = nc.dram_tensor("wo", [E, E], BF16, kind="ExternalInput").ap()
    bq = nc.dram_tensor("bq", [E], F32, kind="ExternalInput").ap()
    bq_pp = nc.dram_tensor("bq_pp", [128, EC], F32, kind="ExternalInput").ap()
    bk_pp = nc.dram_tensor("bk_pp", [128, EC], F32, kind="ExternalInput").ap()
    bv = nc.dram_tensor("bv", [E], F32, kind="ExternalInput").ap()
    bo = nc.dram_tensor("bo", [E], F32, kind="ExternalInput").ap()
    gam = nc.dram_tensor("gam", [E], F32, kind="ExternalInput").ap()
    bet = nc.dram_tensor("bet", [E], F32, kind="ExternalInput").ap()
    out = nc.dram_tensor("out", [NQC, E], F32, kind="ExternalOutput").ap()

    def bcast_row(vec_ap, parts=128):
        return bass.AP(tensor=vec_ap.tensor, offset=vec_ap.offset,
                       ap=[[0, parts], [1, vec_ap.shape[0]]])

    with tile.TileContext(nc) as tc, ExitStack() as ctx:
        # ---- persistent pools -------------------------------------------
        per = ctx.enter_context(tc.tile_pool(name="per", bufs=1))
        dram = ctx.enter_context(tc.tile_pool(name="dram", bufs=1, space="DRAM"))

        v_sb = per.tile([128, KC, H, D + 1], BF16)      # V with ones col
        kpT_sb = per.tile([128, EC, NK], BF16)          # K proj, transposed
        qnT_sb = per.tile([128, EC, NQC], BF16)         # normalized Q, transposed
        aoT_sb = per.tile([128, EC, NQC], BF16)         # attn out, transposed
        rk_pp = per.tile([128, KC], F32)                # 0.125/||k|| per key
        ones128 = per.tile([128, 1], BF16)
        nc.vector.memset(ones128, 1.0)
        nc.vector.memset(v_sb[:, :, :, D:D + 1], 1.0)
        eps24 = per.tile([128, 1], F32)
        nc.vector.memset(eps24, 1e-24)
        epsln = per.tile([128, 1], F32)
        nc.vector.memset(epsln, LN_EPS)
        bk_sb = per.tile([128, EC], F32)
        nc.sync.dma_start(out=bk_sb, in_=bk_pp)
        bq_sb = per.tile([128, EC], F32)
        nc.sync.dma_start(out=bq_sb, in_=bq_pp)

        qp_dram = dram.tile([NQC, E], F32)              # Qp + bo (residual)
        nsq_dram = dram.tile([1, NK], F32)              # ||k||^2 per key
        rq_dram = dram.tile([1, NQC], F32)              # 1/||q|| per query
        rs_dram = dram.tile([H, NQC], F32)              # attn rowsums
        rr_dram = dram.tile([H, NQC], F32)              # 1/rowsum

        # phase-A inputs first on the sync queue (first consumer); B/C
        # inputs on other engine queues so they stream in parallel
        pa_cm = tc.tile_pool(name="pa", bufs=1, side="right")
        pa = pa_cm.__enter__()
        vT_sb = pa.tile([128, IC_K, NK], BF16)
        wv_sb = pa.tile([128, IC_K, E], BF16)
        bv_bc = pa.tile([128, E], F32)
        nc.sync.dma_start(out=wv_sb, in_=wv.rearrange("(c p) e -> p c e", p=128))
        vT_r = vT.rearrange("(c p) n -> p c n", p=128)
        for ks in range(KS):
            nc.sync.dma_start(out=vT_sb[:, :, ks * 512:(ks + 1) * 512],
                              in_=vT_r[:, :, ks * 512:(ks + 1) * 512])
        nc.gpsimd.dma_start(out=bv_bc, in_=bcast_row(bv))

        ldq_cm = tc.tile_pool(name="ldq", bufs=1)
        ldq = ldq_cm.__enter__()
        qT_sb = ldq.tile([128, IC_Q, NQC], BF16)
        wq_sb = ldq.tile([128, IC_Q, E], BF16)
        nc.scalar.dma_start(out=qT_sb, in_=qT.rearrange("(c p) n -> p c n", p=128))
        nc.scalar.dma_start(out=wq_sb, in_=wq.rearrange("(c p) e -> p c e", p=128))

        ldk_cm = tc.tile_pool(name="ldk", bufs=1)
        ldk = ldk_cm.__enter__()
        kT_sb = ldk.tile([128, IC_K, NK], BF16)
        wk_sb = ldk.tile([128, IC_K, E], BF16)
        nc.gpsimd.dma_start(out=kT_sb, in_=kT.rearrange("(c p) n -> p c n", p=128))
        nc.gpsimd.dma_start(out=wk_sb, in_=wk.rearrange("(c p) e -> p c e", p=128))

        # ---- phase A: V = value @ Wv + bv  (natural, +ones col) ---------
        with tc.tile_pool(name="psv", bufs=4, space="PSUM") as psv:
            for kc in range(KC):
                for ec in range(2):
                    ps_v = psv.tile([128, 512], F32)
                    for ic in range(IC_K):
                        nc.tensor.matmul(ps_v,
                                         vT_sb[:, ic, kc * 128:(kc + 1) * 128],
                                         wv_sb[:, ic, ec * 512:(ec + 1) * 512],
                                         start=(ic == 0), stop=(ic == IC_K - 1))
                    nc.vector.tensor_add(
                        out=v_sb[:, kc, ec * 8:(ec + 1) * 8, 0:D],
                        in0=ps_v.rearrange("p (h d) -> p h d", d=D),
                        in1=bv_bc[:, ec * 512:(ec + 1) * 512].rearrange(
                            "p (h d) -> p h d", d=D))
        pa_cm.__exit__(None, None, None)

        # ---- phase C: Qp(+bo) natural -> DRAM; qnT via transposed proj --
        with tc.tile_pool(name="qsc", bufs=2) as qsc, \
             tc.tile_pool(name="psq", bufs=2, space="PSUM") as psq, \
             tc.tile_pool(name="psqt", bufs=4, space="PSUM") as psqt:
            bqo_bc = qsc.tile([128, E], F32, tag="bqo")
            bq_bc = qsc.tile([128, E], F32, tag="bqb")
            nc.gpsimd.dma_start(out=bq_bc, in_=bcast_row(bq))
            nc.gpsimd.dma_start(out=bqo_bc, in_=bcast_row(bo))
            nc.vector.tensor_add(out=bqo_bc, in0=bqo_bc, in1=bq_bc)
            for nt in range(NT):
                ps_q = psq.tile([128, E], F32)
                for half in range(2):
                    for ic in range(IC_Q):
                        nc.tensor.matmul(ps_q[:, half * 512:(half + 1) * 512],
                                         qT_sb[:, ic, nt * 128:(nt + 1) * 128],
                                         wq_sb[:, ic, half * 512:(half + 1) * 512],
                                         start=(ic == 0), stop=(ic == IC_Q - 1))
                # residual written with bq AND bo folded in
                qp_st = qsc.tile([128, E], F32, tag="qpst")
                nc.vector.tensor_add(out=qp_st, in0=ps_q, in1=bqo_bc)
                nc.sync.dma_start(out=qp_dram[nt * 128:(nt + 1) * 128, :],
                                  in_=qp_st)
                # ||q||: from Qp WITHOUT bo
                qp_nb = qsc.tile([128, E], F32, tag="qpnb")
                nc.vector.tensor_add(out=qp_nb, in0=ps_q, in1=bq_bc)
                sq_q = qsc.tile([128, E], F32, tag="sqq")
                nc.vector.tensor_mul(out=sq_q, in0=qp_nb, in1=qp_nb)
                ssq = qsc.tile([128, 1], F32, tag="ssq")
                nc.vector.reduce_sum(out=ssq, in_=sq_q, axis=mybir.AxisListType.X)
                nc.scalar.activation(out=ssq, in_=ssq, func=AF.Sqrt,
                                     bias=eps24, scale=1.0)
                rq_t = qsc.tile([128, 1], F32, tag="rqt")
                nc.vector.reciprocal(out=rq_t, in_=ssq)
                nc.sync.dma_start(
                    out=rq_dram[0:1, nt * 128:(nt + 1) * 128].rearrange(
                        "one p -> p one"),
                    in_=rq_t)
            rq_bc = qsc.tile([128, NQC], F32, tag="rqbc")
            nc.gpsimd.dma_start(
                out=rq_bc,
                in_=bass.AP(tensor=rq_dram.tensor, offset=rq_dram.offset,
                            ap=[[0, 128], [1, NQC]]))
            # transposed projection: qnT[e,q] = (Qp^T + bq) * rq
            for ec in range(EC):
                ps_t = psqt.tile([128, NQC], F32)
                for ic in range(IC_Q):
                    nc.tensor.matmul(ps_t,
                                     wq_sb[:, ic, ec * 128:(ec + 1) * 128],
                                     qT_sb[:, ic, :],
                                     start=(ic == 0), stop=(ic == IC_Q - 1))
                nc.vector.scalar_tensor_tensor(
                    out=qnT_sb[:, ec, :], in0=ps_t,
                    scalar=bq_sb[:, ec:ec + 1], in1=rq_bc,
                    op0=ALU.add, op1=ALU.mult)

        # ---- phase B: K proj (transposed) + key norms, PE-pipelined -----
        with tc.tile_pool(name="pb", bufs=3) as pb, \
             tc.tile_pool(name="psk", bufs=2, space="PSUM") as psk, \
             tc.tile_pool(name="pss", bufs=2, space="PSUM") as pss:

            def norm_chain(ks, ps_ss):
                # emit only after ps_ss's stop matmul: 0.125/sqrt(||k||^2)
                nsq_sb = pb.tile([1, 512], F32, tag="nsq")
                nc.vector.tensor_copy(out=nsq_sb, in_=ps_ss)
                nc.gpsimd.dma_start(out=nsq_dram[:, ks * 512:(ks + 1) * 512],
                                    in_=nsq_sb)
                nsq_pp = pb.tile([128, KC // KS], F32, tag="npp")
                nc.gpsimd.dma_start(
                    out=nsq_pp,
                    in_=nsq_dram[:, ks * 512:(ks + 1) * 512].rearrange(
                        "one (c p) -> p (one c)", p=128))
                nrm = pb.tile([128, KC // KS], F32, tag="nrm")
                nc.scalar.activation(out=nrm, in_=nsq_pp, func=AF.Sqrt,
                                     bias=eps24, scale=1.0)
                nc.vector.reciprocal(out=nrm, in_=nrm)
                kpb = KC // KS
                nc.scalar.mul(out=rk_pp[:, ks * kpb:(ks + 1) * kpb], in_=nrm,
                              mul=SCALE)

            pend = None     # (ps_ss, sq, ec, ks) norm-MM lagging one step
            ss_tiles = {}
            for ks in range(KS):
                ss_tiles[ks] = pss.tile([1, 512], F32, tag="ss",
                                        name=f"ss{ks}")
                for ec in range(EC):
                    ps_k = psk.tile([128, 512], F32)
                    for ic in range(IC_K):
                        nc.tensor.matmul(ps_k,
                                         wk_sb[:, ic, ec * 128:(ec + 1) * 128],
                                         kT_sb[:, ic, ks * 512:(ks + 1) * 512],
                                         start=(ic == 0), stop=(ic == IC_K - 1))
                    if pend is not None:
                        psq_, sq_, ec_, ks_ = pend
                        nc.tensor.matmul(psq_, ones128, sq_,
                                         start=(ec_ == 0), stop=(ec_ == EC - 1))
                        if ec_ == EC - 1:
                            norm_chain(ks_, psq_)
                    kslice = kpT_sb[:, ec, ks * 512:(ks + 1) * 512]
                    nc.vector.tensor_scalar_add(out=kslice, in0=ps_k,
                                                scalar1=bk_sb[:, ec:ec + 1])
                    sq = pb.tile([128, 512], BF16, tag="sq")
                    nc.vector.tensor_mul(out=sq, in0=kslice, in1=kslice)
                    pend = (ss_tiles[ks], sq, ec, ks)
            psq_, sq_, ec_, ks_ = pend
            nc.tensor.matmul(psq_, ones128, sq_,
                             start=(ec_ == 0), stop=(ec_ == EC - 1))
            norm_chain(ks_, psq_)
        ldk_cm.__exit__(None, None, None)
        ldq_cm.__exit__(None, None, None)

        # ---- attention: hp-major, PSUM-accumulated over all 16 kc -------
        # wo + LN params + residual staged here to overlap attention
        lde = ctx.enter_context(tc.tile_pool(name="lde", bufs=1))
        wo_sb = lde.tile([128, EC, E], BF16)
        gam_bc = lde.tile([128, E], F32)
        bet_bc = lde.tile([128, E], F32)
        qp_ld = lde.tile([128, NT, E], F32)
        nc.sync.dma_start(out=wo_sb, in_=wo.rearrange("(c p) e -> p c e", p=128))
        nc.sync.dma_start(out=qp_ld,
                          in_=qp_dram.rearrange("(t p) e -> p t e", p=128))
        nc.gpsimd.dma_start(out=gam_bc, in_=bcast_row(gam))
        nc.gpsimd.dma_start(out=bet_bc, in_=bcast_row(bet))

        with tc.tile_pool(name="esp", bufs=3) as esp, \
             tc.tile_pool(name="pol", bufs=2) as pol, \
             tc.tile_pool(name="aor", bufs=4) as aor, \
             tc.tile_pool(name="nrp", bufs=3) as nrp, \
             tc.tile_pool(name="ps_s", bufs=2, space="PSUM") as ps_sp, \
             tc.tile_pool(name="ps_o", bufs=4, space="PSUM") as ps_op:

            def emit_scores(hp, kc):
                ps_s = ps_sp.tile([128, 2 * NQC], F32, tag="s",
                                  name=f"s{hp}_{kc}")
                for i in range(2):
                    nc.tensor.matmul(
                        ps_s[:, i * NQC:(i + 1) * NQC],
                        kpT_sb[i * D:(i + 1) * D, hp, kc * 128:(kc + 1) * 128],
                        qnT_sb[i * D:(i + 1) * D, hp, :],
                        start=True, stop=True)
                return ps_s

            def emit_exp(ps_s, kc):
                es = esp.tile([128, 2 * NQC], BF16, tag="es")
                if kc % 4 == 3:
                    # DVE polynomial: exp(s) ~ (((s+2)^2+4)/8)^2, |err|<1e-4
                    t = pol.tile([128, 2 * NQC], BF16, tag="t")
                    nc.vector.tensor_scalar(out=t, in0=ps_s,
                                            scalar1=rk_pp[:, kc:kc + 1],
                                            scalar2=2.0,
                                            op0=ALU.mult, op1=ALU.add)
                    h2 = pol.tile([128, 2 * NQC], BF16, tag="h")
                    nc.vector.scalar_tensor_tensor(out=h2, in0=t, scalar=0.5,
                                                   in1=t, op0=ALU.mult,
                                                   op1=ALU.mult)
                    w2 = pol.tile([128, 2 * NQC], BF16, tag="w")
                    nc.vector.tensor_scalar(out=w2, in0=h2, scalar1=0.25,
                                            scalar2=0.5,
                                            op0=ALU.mult, op1=ALU.add)
                    nc.vector.tensor_mul(out=es, in0=w2, in1=w2)
                else:
                    nc.scalar.activation(out=es, in_=ps_s, func=AF.Exp,
                                         scale=rk_pp[:, kc:kc + 1], bias=0.0)
                return es

            po = {}
            ps_pend = emit_scores(0, 0)
            for hp in range(HP):
                po[hp] = [ps_op.tile([D + 1, NQC], F32, tag="po",
                                     name=f"po{hp}_{j}")
                          for j in range(2)]
                for kc in range(KC):
                    es = emit_exp(ps_pend, kc)
                    # emit next scores before av so the in-order PE queue
                    # has independent work while exp runs
                    if kc + 1 < KC:
                        ps_pend = emit_scores(hp, kc + 1)
                    elif hp + 1 < HP:
                        ps_pend = emit_scores(hp + 1, 0)
                    for i in range(2):
                        nc.tensor.matmul(po[hp][i],
                                         v_sb[:, kc, 2 * hp + i, :],
                                         es[:, i * NQC:(i + 1) * NQC],
                                         start=(kc == 0), stop=(kc == KC - 1))

                # free PSUM immediately: copy po -> SBUF staging
                ao_raw = [aor.tile([D + 1, NQC], F32, tag="ao",
                                   name=f"ao{hp}_{j}") for j in range(2)]
                for i in range(2):
                    nc.vector.tensor_copy(out=ao_raw[i], in_=po[hp][i])
                    nc.gpsimd.dma_start(
                        out=rs_dram[2 * hp + i:2 * hp + i + 1, :],
                        in_=ao_raw[i][D:D + 1, :])
                # batched reciprocal of the two rowsums via DRAM round-trip
                rs_pp = nrp.tile([128, 2 * NT], F32, tag="rspp")
                nc.gpsimd.dma_start(
                    out=rs_pp,
                    in_=rs_dram[2 * hp:2 * hp + 2, :].rearrange(
                        "h (c p) -> p (h c)", p=128))
                nc.vector.reciprocal(out=rs_pp, in_=rs_pp)
                nc.gpsimd.dma_start(
                    out=rr_dram[2 * hp:2 * hp + 2, :].rearrange(
                        "h (c p) -> p (h c)", p=128),
                    in_=rs_pp)
                for i in range(2):
                    rbc = nrp.tile([D, NQC], F32, tag=f"rbc{i}")
                    rsrc = rr_dram[2 * hp + i:2 * hp + i + 1, :]
                    nc.gpsimd.dma_start(
                        out=rbc, in_=bass.AP(tensor=rsrc.tensor,
                                             offset=rsrc.offset,
                                             ap=[[0, D], [1, NQC]]))
                    # on GpSimd so the DVE queue never waits on the DMAs
                    nc.gpsimd.tensor_mul(
                        out=aoT_sb[i * D:(i + 1) * D, hp, :],
                        in0=ao_raw[i][0:D, :], in1=rbc)

        # ---- phase E: out proj + residual(+bo) + layernorm --------------
        with tc.tile_pool(name="lnp", bufs=2) as lnp, \
             tc.tile_pool(name="psf", bufs=2, space="PSUM") as psf:
            for nt in range(NT):
                ps_f = psf.tile([128, E], F32)
                for half in range(2):
                    for fc in range(EC):
                        nc.tensor.matmul(ps_f[:, half * 512:(half + 1) * 512],
                                         aoT_sb[:, fc, nt * 128:(nt + 1) * 128],
                                         wo_sb[:, fc, half * 512:(half + 1) * 512],
                                         start=(fc == 0), stop=(fc == EC - 1))
                xs = lnp.tile([128, E], F32, tag="xs")
                nc.vector.tensor_add(out=xs, in0=ps_f, in1=qp_ld[:, nt, :])
                stats = lnp.tile([128, 2, 6], F32, tag="st")
                xs3 = xs.rearrange("p (a b) -> p a b", b=512)
                for sg in range(2):
                    nc.vector.bn_stats(out=stats[:, sg, :], in_=xs3[:, sg, :])
                mv = lnp.tile([128, 2], F32, tag="mv")
                nc.vector.bn_aggr(out=mv, in_=stats)
                rstd = lnp.tile([128, 1], F32, tag="rstd")
                nc.scalar.activation(out=rstd, in_=mv[:, 1:2], func=AF.Sqrt,
                                     bias=epsln, scale=1.0)
                nc.vector.reciprocal(out=rstd, in_=rstd)
                nmr = lnp.tile([128, 1], F32, tag="nmr")
                nc.vector.tensor_mul(out=nmr, in0=mv[:, 0:1], in1=rstd)
                nc.scalar.mul(out=nmr, in_=nmr, mul=-1.0)
                xn = lnp.tile([128, E], F32, tag="xn")
                nc.scalar.activation(out=xn, in_=xs, func=AF.Identity,
                                     scale=rstd, bias=nmr)
                nc.vector.tensor_mul(out=xn, in0=xn, in1=gam_bc)
                ot = lnp.tile([128, E], F32, tag="ot")
                nc.vector.tensor_add(out=ot, in0=xn, in1=bet_bc)
                nc.sync.dma_start(out=out[nt * 128:(nt + 1) * 128, :], in_=ot)

    nc.compile()
    return nc


_NC_CACHE = None
_last_in_maps = None


def _get_nc():
    global _NC_CACHE
    if _NC_CACHE is None:
        _NC_CACHE = build()
    return _NC_CACHE


def kernel(**inputs):
    q = np.asarray(inputs["query"], np.float32)
    k = np.asarray(inputs["key"], np.float32)
    v = np.asarray(inputs["value"], np.float32)
    Wq = np.asarray(inputs["Wq"], np.float32).astype(ml_dtypes.bfloat16)
    Wk = np.asarray(inputs["Wk"], np.float32).astype(ml_dtypes.bfloat16)
    Wv = np.asarray(inputs["Wv"], np.float32).astype(ml_dtypes.bfloat16)
    Wo = np.asarray(inputs["Wo"], np.float32).astype(ml_dtypes.bfloat16)
    bq = np.asarray(inputs["bq"], np.float32)
    bk = np.asarray(inputs["bk"], np.float32)
    bv = np.asarray(inputs["bv"], np.float32)
    bo = np.asarray(inputs["bo"], np.float32)
    gam = np.asarray(inputs["ln_gamma"], np.float32)
    bet = np.asarray(inputs["ln_beta"], np.float32)

    bq_pp = np.ascontiguousarray(bq.reshape(EC, 128).T)
    bk_pp = np.ascontiguousarray(bk.reshape(EC, 128).T)
    kTs = [np.ascontiguousarray(k[b].T.astype(ml_dtypes.bfloat16)) for b in range(B)]
    vTs = [np.ascontiguousarray(v[b].T.astype(ml_dtypes.bfloat16)) for b in range(B)]

    in_maps = []
    for c in range(NC):
        b, r0 = c // 4, (c % 4) * NQC
        qTa = np.ascontiguousarray(q[b, r0:r0 + NQC, :].T.astype(ml_dtypes.bfloat16))
        in_maps.append({
            "qT": qTa, "kT": kTs[b], "vT": vTs[b],
            "wq": Wq, "wk": Wk, "wv": Wv, "wo": Wo,
            "bq": bq, "bq_pp": bq_pp, "bk_pp": bk_pp, "bv": bv, "bo": bo,
            "gam": gam, "bet": bet,
        })

    global _last_in_maps
    _last_in_maps = in_maps
    nc = _get_nc()
    res = bass_utils.run_bass_kernel_spmd(nc, in_maps, core_ids=list(range(NC)))

    out = np.empty((B, NQ, E), np.float32)
    for c in range(NC):
        b, r0 = c // 4, (c % 4) * NQC
        out[b, r0:r0 + NQC, :] = res.results[c]["out"]
    return out
